# revision 1
# baseline (speedup 1.0000x reference)
"""Bass/Tile kernel for causal multi-head attention block (nn_BlankAttention).

Sharding: 8 cores = 2 batches x 4 head-groups (4 heads each).
Each core computes, for its batch b and heads hg*4..hg*4+3:
  qkv projection (transposed layouts), causal attention, partial output
  projection y_part = attn_out @ w_out_slice.  Host sums the 4 partials
  per batch.

All matmuls run in float32r (1 cycle/row on TRN2, ~1.4e-4 max rel err).

Per-core DRAM tensors (all float32 bits, declared float32r for matmul use):
  xt    [2048, 2048]  x[b].T               (dmodel, tok)
  wqk   [2048, 1024]  w_in q/k cols        (dmodel, [q_h0|k_h0|q_h1|k_h1|...])
  wv    [2048,  512]  w_in v cols          (dmodel, [v_h0|v_h1|v_h2|v_h3])
  wout  [ 512, 2048]  w_out rows for the 4 heads (head-major)
  maskt [n_u,  128, 512]  mask tiles, 1.0 = allowed, 0.0 = masked
  ones  [ 128,    1]  all ones
  y     [2048, 2048]  output partial (tok, dmodel)   float32

schedule: list over l-tile i (4 tiles of 512 queries) of list of
  (j, mask_idx) -- key tiles (128 keys) to include; mask_idx -1 = no mask.
"""

import numpy as np
import concourse.bass as bass
import concourse.tile as tile
from concourse import bacc, mybir

S = 2048
DM = 2048
NHL = 4          # heads per core
DH = 128
SCALE = 1.0 / (DH ** 0.5)

F32 = mybir.dt.float32
F32R = mybir.dt.float32r
EXP = mybir.ActivationFunctionType.Exp
LN = mybir.ActivationFunctionType.Ln


def build_nc(schedule, n_masks):
    nc = bacc.Bacc("TRN2", target_bir_lowering=False, debug=False, num_devices=8)
    xt_d = nc.dram_tensor("xt", [DM, S], F32R, kind="ExternalInput").ap()
    wqk_d = nc.dram_tensor("wqk", [DM, 2 * NHL * DH], F32R, kind="ExternalInput").ap()
    wv_d = nc.dram_tensor("wv", [DM, NHL * DH], F32R, kind="ExternalInput").ap()
    wout_d = nc.dram_tensor("wout", [NHL * DH, DM], F32R, kind="ExternalInput").ap()
    maskt_d = nc.dram_tensor("maskt", [n_masks, 128, 512], F32R, kind="ExternalInput").ap()
    ones_d = nc.dram_tensor("ones", [128, 128], F32R, kind="ExternalInput").ap()
    y_d = nc.dram_tensor("y", [S, DM], F32, kind="ExternalOutput").ap()

    with tile.TileContext(nc) as tc:
        with tc.tile_pool(name="persist", bufs=1) as pp:
            qkT = pp.tile([128, 8, S], F32R)      # [dh, (2h+isK), tok]
            V = pp.tile([128, 16, 512], F32R)     # [tok%128, tok//128, vfeat]
            masks = pp.tile([128, n_masks, 512], F32R)
            ones_t = pp.tile([128, 128], F32R)

            # ---- Phase 1+2: projections, streaming x^T in d-quarters ----
            with tc.tile_pool(name="proj", bufs=1) as projp, \
                 tc.tile_pool(name="pps", bufs=1, space="PSUM") as pps:
                for q in range(4):
                    wqk_ts = []
                    for ft in range(8):
                        wqk_t = projp.tile([128, 4, 128], F32R, tag="wqk",
                                           bufs=3, name=f"wqk_q{q}f{ft}")
                        if ft < 2:  # weights for first two f-groups land first
                            nc.sync.dma_start(
                                wqk_t[:],
                                wqk_d[512 * q:512 * (q + 1), 128 * ft:128 * (ft + 1)]
                                .rearrange("(t p) f -> p t f", p=128))
                        wqk_ts.append(wqk_t)
                    # x^T quarter, split into 4 tok-chunks so compute can start early
                    xt_c = []
                    for ch in range(4):
                        xc = projp.tile([128, 4, 512], F32R, tag=f"xt{ch}",
                                        bufs=2, name=f"xt_q{q}c{ch}")
                        nc.sync.dma_start(
                            xc[:],
                            xt_d[512 * q:512 * (q + 1), 512 * ch:512 * (ch + 1)]
                            .rearrange("(t p) s -> p t s", p=128))
                        xt_c.append(xc)
                    wv_t = projp.tile([128, 4, 512], F32R, tag="wv", bufs=2,
                                      name=f"wv_q{q}")
                    nc.sync.dma_start(
                        wv_t[:],
                        wv_d[512 * q:512 * (q + 1), :].rearrange("(t p) f -> p t f", p=128))
                    if q == 0:
                        nc.sync.dma_start(masks[:], maskt_d.rearrange("u p c -> p u c"))
                        nc.sync.dma_start(ones_t[:], ones_d[:])
                    for ft in range(8):
                        wqk_t = wqk_ts[ft]
                        if ft >= 2:
                            nc.sync.dma_start(
                                wqk_t[:],
                                wqk_d[512 * q:512 * (q + 1), 128 * ft:128 * (ft + 1)]
                                .rearrange("(t p) f -> p t f", p=128))
                        for tk in range(4):
                            ps = pps.tile([128, 512], F32, tag="ps", bufs=3)
                            for dq in range(4):
                                nc.tensor.matmul(
                                    ps[:], wqk_t[:, dq, :],
                                    xt_c[tk][:, dq, :],
                                    start=(dq == 0), stop=(dq == 3))
                            dst = qkT[:, ft, 512 * tk:512 * (tk + 1)]
                            if q == 0:
                                nc.vector.tensor_copy(dst, ps[:])
                            else:
                                nc.vector.tensor_add(dst, dst, ps[:])
                    for tt in range(16):
                        ps2 = pps.tile([128, 512], F32, tag="ps2", bufs=3)
                        for dq in range(4):
                            nc.tensor.matmul(
                                ps2[:], xt_c[tt // 4][:, dq, 128 * (tt % 4):128 * (tt % 4 + 1)],
                                wv_t[:, dq, :],
                                start=(dq == 0), stop=(dq == 3))
                        dstv = V[:, tt, :]
                        if q == 0:
                            nc.vector.tensor_copy(dstv, ps2[:])
                        else:
                            nc.vector.tensor_add(dstv, dstv, ps2[:])

            # ---- Phase 3+4: attention with interleaved output projection ----
            from collections import deque
            with tc.tile_pool(name="attn", bufs=1) as ap:
                OT = ap.tile([128, 4, S], F32R)       # [dh, h, tok]
                woutT = ap.tile([128, 4, S], F32R)    # [dh, h, od]
                nc.sync.dma_start(woutT[:], wout_d.rearrange("(f p) o -> p f o", p=128))
                filler = deque()

                def make_group(tt, o):
                    # one output-projection group: y[tok tile tt, od tile o]
                    def g(pool, tag, bufs):
                        yp = pool.tile([128, 512], F32, tag=tag, bufs=bufs,
                                       name=f"yp{tt}_{o}")
                        for h in range(4):
                            nc.tensor.matmul(
                                yp[:], OT[:, h, 128 * tt:128 * (tt + 1)],
                                woutT[:, h, 512 * o:512 * (o + 1)],
                                start=(h == 0), stop=(h == 3))
                        ys = pp.tile([128, 512], F32, tag="ys", bufs=3,
                                     name=f"ys{tt}_{o}")
                        if o % 2 == 0:
                            nc.vector.tensor_copy(ys[:], yp[:])
                        else:
                            nc.scalar.copy(ys[:], yp[:])
                        nc.sync.dma_start(
                            y_d[128 * tt:128 * (tt + 1), 512 * o:512 * (o + 1)],
                            ys[:])
                    return g

                def attn_tile(h, i, s4, o_sbs, aps):
                    js = schedule[i]
                    nj = len(js)
                    oacc = aps.tile([128, 512], F32, tag="oacc", bufs=2, name=f"oacc{h}_{i}")
                    sums = aps.tile([1, 512], F32, tag="sums", bufs=2, name=f"sums{h}_{i}")

                    def scores(idx):
                        # software pipelining: scores for j+1 are issued before
                        # the AV/sums matmuls of j, so exp latency is hidden
                        j, mi, lo = js[idx]
                        sc = aps.tile([128, 512], F32, tag="scyp", bufs=4,
                                      name=f"sc{h}_{i}_{j}")
                        nc.tensor.matmul(
                            sc[:, lo:], qkT[:, 2 * h + 1, 128 * j:128 * (j + 1)],
                            qkT[:, 2 * h, 512 * i + lo:512 * (i + 1)],
                            start=True, stop=True)
                        ex = pp.tile([128, 512], F32R, tag="ex", bufs=5,
                                     name=f"ex{h}_{i}_{j}")
                        nc.scalar.activation(ex[:, lo:], sc[:, lo:], EXP, scale=SCALE)
                        if mi >= 0:
                            nc.vector.tensor_mul(ex[:, lo:], ex[:, lo:], masks[:, mi, lo:])
                        return ex

                    def accum(idx, ex):
                        j, mi, lo = js[idx]
                        nc.tensor.matmul(
                            oacc[:, lo:], V[:, j, 128 * h:128 * (h + 1)], ex[:, lo:],
                            start=(idx == 0), stop=(idx == nj - 1))
                        nc.tensor.matmul(
                            sums[:, lo:], ones_t[:, 0:1], ex[:, lo:],
                            start=(idx == 0), stop=(idx == nj - 1))

                    exs = [scores(0)]
                    for idx in range(nj):
                        if idx + 1 < nj:
                            exs.append(scores(idx + 1))
                        accum(idx, exs[idx])
                        exs[idx] = None
                        # y-projection groups of an earlier l-tile fill PE time
                        # while this tile's exps cook on ScalarE
                        if filler and ((h >= 1 and idx >= 4)
                                       or (i == 0 and h >= 2)):
                            filler.popleft()(aps, "scyp", 4)
                    # Evacuate PSUM accumulators to SBUF right away so the banks
                    # recycle without waiting on the (slow, serial) reciprocal.
                    o_sb = ap.tile([128, 512], F32, tag="o_sb", bufs=5, name=f"osb{h}_{i}")
                    nc.vector.tensor_copy(o_sb[:], oacc[:])
                    nc.vector.tensor_copy(s4[32 * h:32 * h + 1, :], sums[:])
                    o_sbs.append(o_sb)

                def norm_tile(i, s4, o_sbs, aps):
                    # One batched reciprocal per l-tile: a [1,512] DVE reciprocal
                    # is ~3.3us of serial single-lane work that blocks the DVE
                    # queue (and with it the mask-muls feeding the PE); batching
                    # the 4 heads pays that cost once instead of four times.
                    rec = ap.tile([128, 512], F32, tag="rec", bufs=2, name=f"rec{i}")
                    nc.vector.reciprocal(rec[:], s4[:])
                    for h in range(4):
                        # broadcast 1/s across partitions as a K=1 outer product
                        # on the PE (gpsimd partition_broadcast has erratic
                        # multi-us start latency)
                        rtmp = ap.tile([1, 512], F32R, tag="rtmp", bufs=2,
                                       name=f"rtmp{h}_{i}")
                        nc.vector.tensor_copy(rtmp[:], rec[32 * h:32 * h + 1, :])
                        bc = aps.tile([128, 512], F32, tag="sums", bufs=2,
                                      name=f"bc{h}_{i}")
                        nc.tensor.matmul(bc[:], ones_t[0:1, :], rtmp[:],
                                         start=True, stop=True)
                        nc.vector.tensor_mul(
                            OT[:, h, 512 * i:512 * (i + 1)], o_sbs[h][:], bc[:])

                # Descending i: long j-loops first (keeps PE dense); y-projection
                # of l-tile i is spread one group per j through l-tile (i-1)'s
                # attention as PE filler while exps cook.
                with tc.tile_pool(name="aps", bufs=1, space="PSUM") as aps:
                    for i in [3, 2, 1, 0]:
                        s4 = ap.tile([128, 512], F32, tag="s4", bufs=2, name=f"s4_{i}")
                        o_sbs = []
                        for h in range(4):
                            attn_tile(h, i, s4, o_sbs, aps)
                        norm_tile(i, s4, o_sbs, aps)
                        if i > 0:
                            filler.extend(make_group(tt, o)
                                          for tt in range(4 * i, 4 * i + 4)
                                          for o in range(4))
                    # Drain leftovers before the pool closes: they don't depend
                    # on l-tile 0's norm chain, so they cover its latency.
                    while filler:
                        filler.popleft()(aps, "scyp", 4)
                # Tail: l-tile 0's projection in a fresh PSUM scope with deep
                # buffering (attention banks are free now).
                with tc.tile_pool(name="aps2", bufs=1, space="PSUM") as aps2:
                    for tt in range(0, 4):
                        for o in range(4):
                            make_group(tt, o)(aps2, "yp", 6)
    nc.compile()
    return nc


def derive_schedule(mask):
    """mask: [S, S] bool, mask[l, L] True = masked (key L not visible to query l).

    Returns (schedule, mask_tiles):
      schedule[i] = list of (j, mask_idx) for l-tile i; mask_idx -1 = all allowed
      mask_tiles: [n_u, 128, 512] float32, allowed=1.0
    """
    schedule = []
    uniq = {}
    tiles = []
    for i in range(4):
        row = []
        for j in range(16):
            blk = mask[512 * i:512 * (i + 1), 128 * j:128 * (j + 1)]  # [l 512, L 128]
            if blk.all():
                continue  # fully masked -> skip tile
            if not blk.any():
                row.append((j, -1, 0))
                continue
            t = (~blk.T).astype(np.float32)  # [L 128, l 512], allowed=1
            # leading fully-masked columns can be skipped entirely; cap so the
            # matmul free size stays >= 256 (fp32r full-rate condition)
            nz = np.flatnonzero(t.any(axis=0))
            lo = min(int(nz[0]) if len(nz) else 0, 256)
            key = t.tobytes()
            if key not in uniq:
                uniq[key] = len(tiles)
                tiles.append(t)
            row.append((j, uniq[key], lo))
        schedule.append(row)
    if not tiles:
        tiles.append(np.ones((128, 512), np.float32))
    return schedule, np.stack(tiles)


def make_core_inputs(x, w_in, w_out, mask_tiles, b, hg):
    """Inputs for core handling batch b, heads hg*4..hg*4+3."""
    heads = range(hg * 4, hg * 4 + 4)
    xt = np.ascontiguousarray(x[b].T)
    wqk = np.concatenate(
        [w_in[:, h * 384 + o:h * 384 + o + 128] for h in heads for o in (0, 128)],
        axis=1)
    wv = np.concatenate([w_in[:, h * 384 + 256:h * 384 + 384] for h in heads], axis=1)
    wout = np.concatenate([w_out[h * 128:(h + 1) * 128, :] for h in heads], axis=0)
    return {
        "xt": np.ascontiguousarray(xt, np.float32),
        "wqk": np.ascontiguousarray(wqk, np.float32),
        "wv": np.ascontiguousarray(wv, np.float32),
        "wout": np.ascontiguousarray(wout, np.float32),
        "maskt": np.ascontiguousarray(mask_tiles, np.float32),
        "ones": np.ones((128, 128), np.float32),
    }



_CACHE = {}


def _get_nc(schedule, n_masks):
    key = (tuple(tuple(r) for r in schedule), n_masks)
    if key not in _CACHE:
        _CACHE[key] = build_nc(schedule, n_masks)
    return _CACHE[key]


def kernel(x, w_in, w_out, mask):
    """Full-input entry point: shards across 8 NeuronCores (batch x head-group),
    runs the Bass kernel SPMD, and reduces the per-core partial outputs."""
    from concourse import bass_utils
    x = np.ascontiguousarray(np.asarray(x), dtype=np.float32)
    w_in = np.ascontiguousarray(np.asarray(w_in), dtype=np.float32)
    w_out = np.ascontiguousarray(np.asarray(w_out), dtype=np.float32)
    B = x.shape[0]
    m2 = np.asarray(mask).reshape(S, S)
    schedule, mask_tiles = derive_schedule(m2)
    nc = _get_nc(schedule, mask_tiles.shape[0])
    in_maps = [make_core_inputs(x, w_in, w_out, mask_tiles, c // 4, c % 4)
               for c in range(8)]
    res = bass_utils.run_bass_kernel_spmd(nc, in_maps, core_ids=list(range(8)))
    y = np.zeros((B, S, DM), np.float32)
    for c in range(8):
        y[c // 4] += res.results[c]["y"]
    return y



# revision 10
# speedup vs baseline: 1.1782x; 1.1782x over previous
"""Bass/Tile kernel for causal multi-head attention block (nn_BlankAttention).

Sharding: 8 cores = 2 batches x 4 head-groups (4 heads each).
Each core computes, for its batch b and heads hg*4..hg*4+3:
  qkv projection (transposed layouts), causal attention, partial output
  projection y_part = attn_out @ w_out_slice.  Host sums the 4 partials
  per batch.

All matmuls run in float32r (1 cycle/row on TRN2, ~1.4e-4 max rel err).

Per-core DRAM tensors (all float32 bits, declared float32r for matmul use):
  xt    [2048, 2048]  x[b].T               (dmodel, tok)
  wqk   [2048, 1024]  w_in q/k cols        (dmodel, [q_h0|k_h0|q_h1|k_h1|...])
  wv    [2048,  512]  w_in v cols          (dmodel, [v_h0|v_h1|v_h2|v_h3])
  wout  [ 512, 2048]  w_out rows for the 4 heads (head-major)
  maskt [n_u,  128, 512]  mask tiles, 1.0 = allowed, 0.0 = masked
  ones  [ 128,    1]  all ones
  y     [2048, 2048]  output partial (tok, dmodel)   float32

schedule: list over l-tile i (4 tiles of 512 queries) of list of
  (j, mask_idx) -- key tiles (128 keys) to include; mask_idx -1 = no mask.
"""

import numpy as np
import ml_dtypes
import concourse.bass as bass
import concourse.tile as tile
from concourse import bacc, mybir

S = 2048
DM = 2048
NHL = 4          # heads per core
DH = 128
SCALE = 1.0 / (DH ** 0.5)

F32 = mybir.dt.float32
F32R = mybir.dt.float32r
BF16 = mybir.dt.bfloat16
NPBF16 = ml_dtypes.bfloat16
EXP = mybir.ActivationFunctionType.Exp
LN = mybir.ActivationFunctionType.Ln


def build_nc(schedule, n_masks):
    nc = bacc.Bacc("TRN2", target_bir_lowering=False, debug=False, num_devices=8)
    xt_d = nc.dram_tensor("xt", [DM, S], BF16, kind="ExternalInput").ap()
    wqk_d = nc.dram_tensor("wqk", [DM, 2 * NHL * DH], BF16, kind="ExternalInput").ap()
    wv_d = nc.dram_tensor("wv", [DM, NHL * DH], BF16, kind="ExternalInput").ap()
    wout_d = nc.dram_tensor("wout", [NHL * DH, DM], BF16, kind="ExternalInput").ap()
    maskt_d = nc.dram_tensor("maskt", [n_masks, 128, 512], BF16, kind="ExternalInput").ap()
    ones_d = nc.dram_tensor("ones", [128, 128], BF16, kind="ExternalInput").ap()
    onesr_d = nc.dram_tensor("onesr", [1, 128], F32R, kind="ExternalInput").ap()
    y_d = nc.dram_tensor("y", [S, DM], F32, kind="ExternalOutput").ap()

    with tile.TileContext(nc) as tc:
        with tc.tile_pool(name="persist", bufs=1) as pp:
            qkT = pp.tile([128, 8, S], BF16)      # [dh, (2h+isK), tok]
            V = pp.tile([128, 16, 512], BF16)     # [tok%128, tok//128, vfeat]
            masks = pp.tile([128, n_masks, 512], BF16)
            ones_t = pp.tile([128, 128], BF16)
            onesr_t = pp.tile([1, 128], F32R)

            # ---- Phase 1+2: projections, streaming x^T in d-quarters ----
            with tc.tile_pool(name="proj", bufs=1) as projp, \
                 tc.tile_pool(name="pps", bufs=1, space="PSUM") as pps:
                for q in range(4):
                    wqk_ts = []
                    for ft in range(8):
                        wqk_t = projp.tile([128, 4, 128], BF16, tag="wqk",
                                           bufs=3, name=f"wqk_q{q}f{ft}")
                        if ft < 2:  # weights for first two f-groups land first
                            nc.sync.dma_start(
                                wqk_t[:],
                                wqk_d[512 * q:512 * (q + 1), 128 * ft:128 * (ft + 1)]
                                .rearrange("(t p) f -> p t f", p=128))
                        wqk_ts.append(wqk_t)
                    # x^T quarter, split into 4 tok-chunks so compute can start early
                    xt_c = []
                    for ch in range(4):
                        xc = projp.tile([128, 4, 512], BF16, tag=f"xt{ch}",
                                        bufs=2, name=f"xt_q{q}c{ch}")
                        nc.sync.dma_start(
                            xc[:],
                            xt_d[512 * q:512 * (q + 1), 512 * ch:512 * (ch + 1)]
                            .rearrange("(t p) s -> p t s", p=128))
                        xt_c.append(xc)
                    wv_t = projp.tile([128, 4, 512], BF16, tag="wv", bufs=2,
                                      name=f"wv_q{q}")
                    nc.sync.dma_start(
                        wv_t[:],
                        wv_d[512 * q:512 * (q + 1), :].rearrange("(t p) f -> p t f", p=128))
                    if q == 0:
                        nc.sync.dma_start(masks[:], maskt_d.rearrange("u p c -> p u c"))
                        nc.sync.dma_start(ones_t[:], ones_d[:])
                        nc.sync.dma_start(onesr_t[:], onesr_d[:])
                    for ft in range(8):
                        wqk_t = wqk_ts[ft]
                        if ft >= 2:
                            nc.sync.dma_start(
                                wqk_t[:],
                                wqk_d[512 * q:512 * (q + 1), 128 * ft:128 * (ft + 1)]
                                .rearrange("(t p) f -> p t f", p=128))
                        for tk in range(4):
                            ps = pps.tile([128, 512], F32, tag="ps", bufs=3)
                            for dq in range(4):
                                nc.tensor.matmul(
                                    ps[:], wqk_t[:, dq, :],
                                    xt_c[tk][:, dq, :],
                                    start=(dq == 0), stop=(dq == 3))
                            dst = qkT[:, ft, 512 * tk:512 * (tk + 1)]
                            if q == 0:
                                nc.vector.tensor_copy(dst, ps[:])
                            else:
                                nc.vector.tensor_add(dst, dst, ps[:])
                    for tt in range(16):
                        ps2 = pps.tile([128, 512], F32, tag="ps2", bufs=3)
                        for dq in range(4):
                            nc.tensor.matmul(
                                ps2[:], xt_c[tt // 4][:, dq, 128 * (tt % 4):128 * (tt % 4 + 1)],
                                wv_t[:, dq, :],
                                start=(dq == 0), stop=(dq == 3))
                        dstv = V[:, tt, :]
                        if q == 0:
                            nc.vector.tensor_copy(dstv, ps2[:])
                        else:
                            nc.vector.tensor_add(dstv, dstv, ps2[:])

            # ---- Phase 3+4: attention with interleaved output projection ----
            from collections import deque
            with tc.tile_pool(name="attn", bufs=1) as ap:
                OT = ap.tile([128, 4, S], BF16)       # [dh, h, tok]
                woutT = ap.tile([128, 4, S], BF16)    # [dh, h, od]
                nc.sync.dma_start(woutT[:], wout_d.rearrange("(f p) o -> p f o", p=128))
                filler = deque()

                def make_group(tt, o):
                    # one output-projection group: y[tok tile tt, od tile o]
                    def g(pool, tag, bufs):
                        yp = pool.tile([128, 512], F32, tag=tag, bufs=bufs,
                                       name=f"yp{tt}_{o}")
                        for h in range(4):
                            nc.tensor.matmul(
                                yp[:], OT[:, h, 128 * tt:128 * (tt + 1)],
                                woutT[:, h, 512 * o:512 * (o + 1)],
                                start=(h == 0), stop=(h == 3))
                        ys = pp.tile([128, 512], F32, tag="ys", bufs=3,
                                     name=f"ys{tt}_{o}")
                        if o % 2 == 0:
                            nc.vector.tensor_copy(ys[:], yp[:])
                        else:
                            nc.scalar.copy(ys[:], yp[:])
                        nc.sync.dma_start(
                            y_d[128 * tt:128 * (tt + 1), 512 * o:512 * (o + 1)],
                            ys[:])
                    return g

                def attn_tile(h, i, s4, o_sbs, aps):
                    js = schedule[i]
                    nj = len(js)
                    oacc = aps.tile([128, 512], F32, tag="oacc", bufs=2, name=f"oacc{h}_{i}")
                    sums = aps.tile([1, 512], F32, tag="sums", bufs=2, name=f"sums{h}_{i}")

                    def scores(idx):
                        # software pipelining: scores for j+1 are issued before
                        # the AV/sums matmuls of j, so exp latency is hidden
                        j, mi, lo = js[idx]
                        sc = aps.tile([128, 512], F32, tag="scyp", bufs=4,
                                      name=f"sc{h}_{i}_{j}")
                        nc.tensor.matmul(
                            sc[:, lo:], qkT[:, 2 * h + 1, 128 * j:128 * (j + 1)],
                            qkT[:, 2 * h, 512 * i + lo:512 * (i + 1)],
                            start=True, stop=True)
                        ex = pp.tile([128, 512], BF16, tag="ex", bufs=5,
                                     name=f"ex{h}_{i}_{j}")
                        nc.scalar.activation(ex[:, lo:], sc[:, lo:], EXP, scale=SCALE)
                        if mi >= 0:
                            nc.vector.tensor_mul(ex[:, lo:], ex[:, lo:], masks[:, mi, lo:])
                        return ex

                    def accum(idx, ex):
                        j, mi, lo = js[idx]
                        nc.tensor.matmul(
                            oacc[:, lo:], V[:, j, 128 * h:128 * (h + 1)], ex[:, lo:],
                            start=(idx == 0), stop=(idx == nj - 1))
                        nc.tensor.matmul(
                            sums[:, lo:], ones_t[:, 0:1], ex[:, lo:],
                            start=(idx == 0), stop=(idx == nj - 1))

                    exs = [scores(0)]
                    for idx in range(nj):
                        if idx + 1 < nj:
                            exs.append(scores(idx + 1))
                        accum(idx, exs[idx])
                        exs[idx] = None
                        # y-projection groups of an earlier l-tile fill PE time
                        # while this tile's exps cook on ScalarE
                        if filler and ((h >= 1 and idx >= 4)
                                       or (i == 0 and h >= 2)):
                            filler.popleft()(aps, "scyp", 4)
                    # Evacuate PSUM accumulators to SBUF right away so the banks
                    # recycle without waiting on the (slow, serial) reciprocal.
                    o_sb = ap.tile([128, 512], F32, tag="o_sb", bufs=5, name=f"osb{h}_{i}")
                    nc.vector.tensor_copy(o_sb[:], oacc[:])
                    nc.vector.tensor_copy(s4[32 * h:32 * h + 1, :], sums[:])
                    o_sbs.append(o_sb)

                def norm_tile(i, s4, o_sbs, aps):
                    # One batched reciprocal per l-tile: a [1,512] DVE reciprocal
                    # is ~3.3us of serial single-lane work that blocks the DVE
                    # queue (and with it the mask-muls feeding the PE); batching
                    # the 4 heads pays that cost once instead of four times.
                    rec = ap.tile([128, 512], F32, tag="rec", bufs=2, name=f"rec{i}")
                    nc.vector.reciprocal(rec[:], s4[:])
                    for h in range(4):
                        # broadcast 1/s across partitions as a K=1 outer product
                        # on the PE (gpsimd partition_broadcast has erratic
                        # multi-us start latency)
                        rtmp = ap.tile([1, 512], F32R, tag="rtmp", bufs=2,
                                       name=f"rtmp{h}_{i}")
                        nc.vector.tensor_copy(rtmp[:], rec[32 * h:32 * h + 1, :])
                        bc = aps.tile([128, 512], F32, tag="sums", bufs=2,
                                      name=f"bc{h}_{i}")
                        nc.tensor.matmul(bc[:], onesr_t[:], rtmp[:],
                                         start=True, stop=True)
                        nc.vector.tensor_mul(
                            OT[:, h, 512 * i:512 * (i + 1)], o_sbs[h][:], bc[:])

                # Descending i: long j-loops first (keeps PE dense); y-projection
                # of l-tile i is spread one group per j through l-tile (i-1)'s
                # attention as PE filler while exps cook.
                with tc.tile_pool(name="aps", bufs=1, space="PSUM") as aps:
                    for i in [3, 2, 1, 0]:
                        s4 = ap.tile([128, 512], F32, tag="s4", bufs=2, name=f"s4_{i}")
                        o_sbs = []
                        for h in range(4):
                            attn_tile(h, i, s4, o_sbs, aps)
                        norm_tile(i, s4, o_sbs, aps)
                        if i > 0:
                            filler.extend(make_group(tt, o)
                                          for tt in range(4 * i, 4 * i + 4)
                                          for o in range(4))
                    # Drain leftovers before the pool closes: they don't depend
                    # on l-tile 0's norm chain, so they cover its latency.
                    while filler:
                        filler.popleft()(aps, "scyp", 4)
                # Tail: l-tile 0's projection in a fresh PSUM scope with deep
                # buffering (attention banks are free now).
                with tc.tile_pool(name="aps2", bufs=1, space="PSUM") as aps2:
                    for tt in range(0, 4):
                        for o in range(4):
                            make_group(tt, o)(aps2, "yp", 6)
    nc.compile()
    return nc


def derive_schedule(mask):
    """mask: [S, S] bool, mask[l, L] True = masked (key L not visible to query l).

    Returns (schedule, mask_tiles):
      schedule[i] = list of (j, mask_idx) for l-tile i; mask_idx -1 = all allowed
      mask_tiles: [n_u, 128, 512] float32, allowed=1.0
    """
    schedule = []
    uniq = {}
    tiles = []
    for i in range(4):
        row = []
        for j in range(16):
            blk = mask[512 * i:512 * (i + 1), 128 * j:128 * (j + 1)]  # [l 512, L 128]
            if blk.all():
                continue  # fully masked -> skip tile
            if not blk.any():
                row.append((j, -1, 0))
                continue
            t = (~blk.T).astype(np.float32)  # [L 128, l 512], allowed=1
            # leading fully-masked columns can be skipped entirely; cap so the
            # matmul free size stays >= 256 (fp32r full-rate condition)
            nz = np.flatnonzero(t.any(axis=0))
            lo = min(int(nz[0]) if len(nz) else 0, 256)
            key = t.tobytes()
            if key not in uniq:
                uniq[key] = len(tiles)
                tiles.append(t)
            row.append((j, uniq[key], lo))
        schedule.append(row)
    if not tiles:
        tiles.append(np.ones((128, 512), np.float32))
    return schedule, np.stack(tiles)


def make_core_inputs(x, w_in, w_out, mask_tiles, b, hg):
    """Inputs for core handling batch b, heads hg*4..hg*4+3."""
    heads = range(hg * 4, hg * 4 + 4)
    xt = np.ascontiguousarray(x[b].T)
    wqk = np.concatenate(
        [w_in[:, h * 384 + o:h * 384 + o + 128] for h in heads for o in (0, 128)],
        axis=1)
    wv = np.concatenate([w_in[:, h * 384 + 256:h * 384 + 384] for h in heads], axis=1)
    wout = np.concatenate([w_out[h * 128:(h + 1) * 128, :] for h in heads], axis=0)
    return {
        "xt": np.ascontiguousarray(xt).astype(NPBF16),
        "wqk": np.ascontiguousarray(wqk).astype(NPBF16),
        "wv": np.ascontiguousarray(wv).astype(NPBF16),
        "wout": np.ascontiguousarray(wout).astype(NPBF16),
        "maskt": np.ascontiguousarray(mask_tiles).astype(NPBF16),
        "ones": np.ones((128, 128), NPBF16),
        "onesr": np.ones((1, 128), np.float32),
    }



_CACHE = {}


def _get_nc(schedule, n_masks):
    key = (tuple(tuple(r) for r in schedule), n_masks)
    if key not in _CACHE:
        _CACHE[key] = build_nc(schedule, n_masks)
    return _CACHE[key]


def kernel(x, w_in, w_out, mask):
    """Full-input entry point: shards across 8 NeuronCores (batch x head-group),
    runs the Bass kernel SPMD, and reduces the per-core partial outputs."""
    from concourse import bass_utils
    x = np.ascontiguousarray(np.asarray(x), dtype=np.float32)
    w_in = np.ascontiguousarray(np.asarray(w_in), dtype=np.float32)
    w_out = np.ascontiguousarray(np.asarray(w_out), dtype=np.float32)
    B = x.shape[0]
    m2 = np.asarray(mask).reshape(S, S)
    schedule, mask_tiles = derive_schedule(m2)
    nc = _get_nc(schedule, mask_tiles.shape[0])
    in_maps = [make_core_inputs(x, w_in, w_out, mask_tiles, c // 4, c % 4)
               for c in range(8)]
    res = bass_utils.run_bass_kernel_spmd(nc, in_maps, core_ids=list(range(8)))
    y = np.zeros((B, S, DM), np.float32)
    for c in range(8):
        y[c // 4] += res.results[c]["y"]
    return y



# revision 12
# speedup vs baseline: 1.1831x; 1.0042x over previous
"""Bass/Tile kernel v3 for causal MHA block (nn_BlankAttention), bf16 matmuls.

Sharding: 8 cores = 2 batches x 4 head-groups (4 heads each).

Structure per core:
  Phase A/B (per tok-chunk tk): QK projection (8 ft groups x 16-deep PSUM
    accumulation) + V projection (4 tt groups) -- x streamed once, dense PE.
  Phase C: attention per l-tile i ascending, 4 heads j-synchronized; softmax
    denominators via col-tiled M=1 matmuls (4 heads concurrent in one PSUM
    bank); y-projection groups of normed l-tiles fill PE during exp stalls.
  Tail: last l-tile's norm (fast reciprocal) + remaining y-proj groups.
"""

import numpy as np
import ml_dtypes
import concourse.bass as bass
import concourse.tile as tile
from concourse import bacc, mybir

S = 2048
DM = 2048
NHL = 4          # heads per core
DH = 128
SCALE = 1.0 / (DH ** 0.5)

F32 = mybir.dt.float32
F32R = mybir.dt.float32r
BF16 = mybir.dt.bfloat16
NPBF16 = ml_dtypes.bfloat16
EXP = mybir.ActivationFunctionType.Exp


def build_nc(schedule, n_masks):
    nc = bacc.Bacc("TRN2", target_bir_lowering=False, debug=False, num_devices=8)
    xt_d = nc.dram_tensor("xt", [DM, S], BF16, kind="ExternalInput").ap()
    wqk_d = nc.dram_tensor("wqk", [DM, 2 * NHL * DH], BF16, kind="ExternalInput").ap()
    wv_d = nc.dram_tensor("wv", [DM, NHL * DH], BF16, kind="ExternalInput").ap()
    wout_d = nc.dram_tensor("wout", [NHL * DH, DM], BF16, kind="ExternalInput").ap()
    maskt_d = nc.dram_tensor("maskt", [n_masks, 128, 512], BF16, kind="ExternalInput").ap()
    ones_d = nc.dram_tensor("ones", [128, 128], BF16, kind="ExternalInput").ap()
    onesr_d = nc.dram_tensor("onesr", [1, 128], F32R, kind="ExternalInput").ap()
    y_d = nc.dram_tensor("y", [S, DM], F32, kind="ExternalOutput").ap()

    with tile.TileContext(nc) as tc:
        with tc.tile_pool(name="persist", bufs=1) as pp:
            qkT = pp.tile([128, 8, S], BF16)      # [dh, (2h+isK), tok]
            V = pp.tile([128, 16, 512], BF16)     # [tok%128, tok//128, vfeat]
            masks = pp.tile([128, n_masks, 512], BF16)
            ones_t = pp.tile([128, 128], BF16)
            onesr_t = pp.tile([1, 128], F32R)
            OT = pp.tile([128, 4, S], BF16)       # [dh, h, tok]
            woutT = pp.tile([128, 4, S], BF16)    # [dh, h, od]

            # ---- Phase A/B: QK + V projections, x streamed once ----
            with tc.tile_pool(name="proj", bufs=1) as projp, \
                 tc.tile_pool(name="pps", bufs=1, space="PSUM") as pps:
                wqk_fts = []
                for ft in range(2):
                    w = projp.tile([128, 16, 128], BF16, tag="wqk", bufs=8,
                                   name=f"wqk{ft}")
                    nc.sync.dma_start(
                        w[:], wqk_d[:, 128 * ft:128 * (ft + 1)]
                        .rearrange("(t p) f -> p t f", p=128))
                    wqk_fts.append(w)
                xcs = []
                for tk in range(4):
                    xc = projp.tile([128, 16, 512], BF16, tag="xt", bufs=2,
                                    name=f"xt{tk}")
                    if tk == 0:
                        for qq in range(4):
                            nc.sync.dma_start(
                                xc[:, 4 * qq:4 * (qq + 1), :],
                                xt_d[512 * qq:512 * (qq + 1), 0:512]
                                .rearrange("(t p) s -> p t s", p=128))
                    xcs.append(xc)
                for ft in range(2, 8):
                    w = projp.tile([128, 16, 128], BF16, tag="wqk", bufs=8,
                                   name=f"wqk{ft}")
                    nc.sync.dma_start(
                        w[:], wqk_d[:, 128 * ft:128 * (ft + 1)]
                        .rearrange("(t p) f -> p t f", p=128))
                    wqk_fts.append(w)
                wv_sb = projp.tile([128, 16, 512], BF16, tag="wv", bufs=1)
                nc.sync.dma_start(wv_sb[:], wv_d.rearrange("(t p) f -> p t f", p=128))
                nc.sync.dma_start(masks[:], maskt_d.rearrange("u p c -> p u c"))
                nc.sync.dma_start(ones_t[:], ones_d[:])
                nc.sync.dma_start(onesr_t[:], onesr_d[:])
                nc.sync.dma_start(woutT[:], wout_d.rearrange("(f p) o -> p f o", p=128))

                for tk in range(4):
                    xc = xcs[tk]
                    if tk > 0:
                        for qq in range(4):
                            nc.sync.dma_start(
                                xc[:, 4 * qq:4 * (qq + 1), :],
                                xt_d[512 * qq:512 * (qq + 1),
                                     512 * tk:512 * (tk + 1)]
                                .rearrange("(t p) s -> p t s", p=128))
                    for ft in range(8):
                        ps = pps.tile([128, 512], F32, tag="ps", bufs=4,
                                      name=f"ps{tk}_{ft}")
                        for dq in range(16):
                            nc.tensor.matmul(
                                ps[:], wqk_fts[ft][:, dq, :], xc[:, dq, :],
                                start=(dq == 0), stop=(dq == 15))
                        nc.vector.tensor_copy(qkT[:, ft, 512 * tk:512 * (tk + 1)],
                                              ps[:])
                    for tl in range(4):
                        tt = 4 * tk + tl
                        ps2 = pps.tile([128, 512], F32, tag="ps", bufs=4,
                                       name=f"psv{tt}")
                        for dq in range(16):
                            nc.tensor.matmul(
                                ps2[:], xc[:, dq, 128 * tl:128 * (tl + 1)],
                                wv_sb[:, dq, :],
                                start=(dq == 0), stop=(dq == 15))
                        nc.scalar.copy(V[:, tt, :], ps2[:])

            # ---- Phase C: attention + interleaved output projection ----
            from collections import deque
            filler = deque()
            with tc.tile_pool(name="attn", bufs=1) as ap, \
                 tc.tile_pool(name="aps", bufs=1, space="PSUM") as aps:
                oacc4 = aps.tile([128, 4, 512], F32)   # AV accum, banks 0-3
                s4 = aps.tile([128, 512], F32)         # sums rows 32h, bank 4

                def make_group(tt, o, on_act):
                    def g():
                        yp = aps.tile([128, 512], F32, tag="misc", bufs=1,
                                      name=f"yp{tt}_{o}")
                        for h in range(4):
                            nc.tensor.matmul(
                                yp[:], OT[:, h, 128 * tt:128 * (tt + 1)],
                                woutT[:, h, 512 * o:512 * (o + 1)],
                                start=(h == 0), stop=(h == 3))
                        ys = ap.tile([128, 512], F32, tag="ys", bufs=3,
                                     name=f"ys{tt}_{o}")
                        if on_act:
                            nc.scalar.copy(ys[:], yp[:])
                        else:
                            nc.vector.tensor_copy(ys[:], yp[:])
                        nc.sync.dma_start(
                            y_d[128 * tt:128 * (tt + 1), 512 * o:512 * (o + 1)],
                            ys[:])
                    return g

                reserve = []
                for i in range(4):
                    js = schedule[i]
                    nj = len(js)
                    for jidx, (j, mi, lo) in enumerate(js):
                        exs = []
                        for h in range(4):
                            sc = aps.tile([128, 512], F32, tag="sc", bufs=2,
                                          name=f"sc{i}_{j}_{h}")
                            nc.tensor.matmul(
                                sc[:, lo:], qkT[:, 2 * h + 1, 128 * j:128 * (j + 1)],
                                qkT[:, 2 * h, 512 * i + lo:512 * (i + 1)],
                                start=True, stop=True)
                            ex = ap.tile([128, 512], BF16, tag="ex", bufs=10,
                                         name=f"ex{i}_{j}_{h}")
                            nc.scalar.activation(ex[:, lo:], sc[:, lo:], EXP,
                                                 scale=SCALE)
                            if mi >= 0:
                                nc.vector.tensor_mul(ex[:, lo:], ex[:, lo:],
                                                     masks[:, mi, lo:])
                            exs.append(ex)
                        for h in range(4):
                            nc.tensor.matmul(
                                oacc4[:, h, lo:], V[:, j, 128 * h:128 * (h + 1)],
                                exs[h][:, lo:],
                                start=(jidx == 0), stop=(jidx == nj - 1))
                            # surplus-paced extra filler slot
                            if h == 1 and len(filler) > 2 * (nj - jidx):
                                filler.popleft()()
                        for h in range(4):
                            nc.tensor.matmul(
                                s4[32 * h:32 * h + 1, lo:], ones_t[:, 0:1],
                                exs[h][:, lo:],
                                start=(jidx == 0),
                                stop=(jidx == nj - 1),
                                tile_position=(0, 32 * h),
                                skip_group_check=True)
                        if filler:
                            filler.popleft()()

                    # reserved groups cover the last norm's latency window
                    if i == 3:
                        while filler:
                            filler.popleft()()
                        for u in reserve:
                            u()

                    o_sbs = []
                    for h in range(4):
                        o_sb = ap.tile([128, 512], F32, tag="osb", bufs=5,
                                       name=f"osb{i}_{h}")
                        nc.vector.tensor_copy(o_sb[:], oacc4[:, h, :])
                        o_sbs.append(o_sb)
                    rec = ap.tile([128, 512], F32, tag="rec", bufs=2,
                                  name=f"rec{i}")
                    nc.vector.reciprocal_approx_fast(rec[:], s4[:])
                    for h in range(4):
                        rtmp = ap.tile([1, 512], F32R, tag="rtmp", bufs=2,
                                       name=f"rtmp{i}_{h}")
                        nc.vector.tensor_copy(rtmp[:], rec[32 * h:32 * h + 1, :])
                        bc = aps.tile([128, 512], F32, tag="misc", bufs=1,
                                      name=f"bc{i}_{h}")
                        nc.tensor.matmul(bc[:], onesr_t[:], rtmp[:],
                                         start=True, stop=True)
                        nc.vector.tensor_mul(
                            OT[:, h, 512 * i:512 * (i + 1)], o_sbs[h][:], bc[:])

                    if i < 2:
                        filler.extend(make_group(tt, o, False)
                                      for tt in range(4 * i, 4 * i + 4)
                                      for o in range(4))
                    elif i == 2:
                        units = [make_group(tt, o, False)
                                 for tt in range(8, 12) for o in range(4)]
                        filler.extend(units[:8])
                        reserve = units[8:]
                    else:
                        # tail: last l-tile's groups, copies split ACT/DVE
                        for tt in range(12, 16):
                            for o in range(4):
                                make_group(tt, o, o % 2 == 0)()
    nc.compile()
    return nc


def derive_schedule(mask):
    """mask: [S, S] bool, True = masked. Returns (schedule, mask_tiles)."""
    schedule = []
    uniq = {}
    tiles = []
    for i in range(4):
        row = []
        for j in range(16):
            blk = mask[512 * i:512 * (i + 1), 128 * j:128 * (j + 1)]
            if blk.all():
                continue
            if not blk.any():
                row.append((j, -1, 0))
                continue
            t = (~blk.T).astype(np.float32)  # [L 128, l 512], allowed=1
            nz = np.flatnonzero(t.any(axis=0))
            lo = min(int(nz[0]) if len(nz) else 0, 256)
            key = t.tobytes()
            if key not in uniq:
                uniq[key] = len(tiles)
                tiles.append(t)
            row.append((j, uniq[key], lo))
        schedule.append(row)
    if not tiles:
        tiles.append(np.ones((128, 512), np.float32))
    return schedule, np.stack(tiles)


def make_core_inputs(x, w_in, w_out, mask_tiles, b, hg):
    """Inputs for core handling batch b, heads hg*4..hg*4+3."""
    heads = range(hg * 4, hg * 4 + 4)
    xt = np.ascontiguousarray(x[b].T)
    wqk = np.concatenate(
        [w_in[:, h * 384 + o:h * 384 + o + 128] for h in heads for o in (0, 128)],
        axis=1)
    wv = np.concatenate([w_in[:, h * 384 + 256:h * 384 + 384] for h in heads], axis=1)
    wout = np.concatenate([w_out[h * 128:(h + 1) * 128, :] for h in heads], axis=0)
    return {
        "xt": np.ascontiguousarray(xt).astype(NPBF16),
        "wqk": np.ascontiguousarray(wqk).astype(NPBF16),
        "wv": np.ascontiguousarray(wv).astype(NPBF16),
        "wout": np.ascontiguousarray(wout).astype(NPBF16),
        "maskt": np.ascontiguousarray(mask_tiles).astype(NPBF16),
        "ones": np.ones((128, 128), NPBF16),
        "onesr": np.ones((1, 128), np.float32),
    }


_CACHE = {}


def _get_nc(schedule, n_masks):
    key = (tuple(tuple(r) for r in schedule), n_masks)
    if key not in _CACHE:
        _CACHE[key] = build_nc(schedule, n_masks)
    return _CACHE[key]


def kernel(x, w_in, w_out, mask):
    """Full-input entry point: shards across 8 NeuronCores (batch x head-group),
    runs the Bass kernel SPMD, and reduces the per-core partial outputs."""
    from concourse import bass_utils
    x = np.ascontiguousarray(np.asarray(x), dtype=np.float32)
    w_in = np.ascontiguousarray(np.asarray(w_in), dtype=np.float32)
    w_out = np.ascontiguousarray(np.asarray(w_out), dtype=np.float32)
    B = x.shape[0]
    m2 = np.asarray(mask).reshape(S, S)
    schedule, mask_tiles = derive_schedule(m2)
    nc = _get_nc(schedule, mask_tiles.shape[0])
    in_maps = [make_core_inputs(x, w_in, w_out, mask_tiles, c // 4, c % 4)
               for c in range(8)]
    res = bass_utils.run_bass_kernel_spmd(nc, in_maps, core_ids=list(range(8)))
    y = np.zeros((B, S, DM), np.float32)
    for c in range(8):
        y[c // 4] += res.results[c]["y"]
    return y
